# revision 13
# baseline (speedup 1.0000x reference)
"""CrossAttention TRN2 kernel.

Problem (hardcoded shapes):
  x    [4, 2048, 1024], cond [4, 2048, 1024]
  Wq/Wk/Wv [1024, 1024], Wo [1024, 1024], bo [1024]
  out = softmax((x@Wq) reshaped to 8 heads of 128 @ (cond@Wk)^T * 0.125) @ (cond@Wv) @ Wo + bo

Sharding: 8 cores = (batch b in 0..3) x (query-half ih in 0..1).
Each core computes 1024 query rows for one batch, all 8 heads; K/V projection
for that batch is replicated across the 2 cores sharing it. No collectives.

Device layouts (host pre-transposes and pre-casts to bf16):
  xT    [1024 cdim, 1024 i]   = x[b, ih*1024:(ih+1)*1024, :].T
  condT [1024 cdim, 2048 j]   = cond[b].T
  Weights as-is (Wq pre-scaled by 0.125). All matmuls run in bf16
  (fp32r moving operands stream at ~2 cycles/row and DVFS-throttle the PE).

Schedule notes:
  - V projected for all 8 heads upfront (free-512 matmuls).
  - Per-group Wq/Wk slices double-buffered and prefetched one group ahead
    so the PE never waits on weight DMA at group boundaries.
  - The two heads of a group are interleaved at the key-tile level so the
    softmax-denominator drain of one head hides under the other's matmuls,
    and consecutive matmul pairs share their stationary operand.
  - PSUM: tag "sc" ([128,1024] f32, bufs=2, also used by projection/out
    accumulators) + tag "av" ([128,512] f32, bufs=4) = exactly 8 banks.
  - Softmax denominator accumulates in bf16 on DVE (2x rate), partition
    all-reduce on gpsimd, fast-approx reciprocal on DVE.
"""
import numpy as np
import ml_dtypes

import concourse.bass as bass
import concourse.bacc as bacc
import concourse.tile as tile
from concourse import bass_isa, mybir
from concourse.bass_utils import run_bass_kernel_spmd

F32 = mybir.dt.float32
BF16 = mybir.dt.bfloat16
EXP = mybir.ActivationFunctionType.Exp

B, NQ, NK, D = 4, 2048, 2048, 1024   # D = query_dim = cond_dim = inner_dim = out_dim
H, DH = 8, 128                        # heads, per-head dim
SCALE = 64 ** -0.5                    # reference uses dim_head=64 for the scale
NCORES = 8
IQ = NQ // 2                          # query rows per core (1024)
KT = D // 128                         # contraction tiles (8)
GROUPS, HPG = 4, 2                    # head groups of 2 heads
JT = NK // 128                        # key tiles (16)


def build_nc():
    nc = bacc.Bacc()
    xT = nc.declare_dram_parameter("xT", [D, IQ], BF16, isOutput=False)
    condT = nc.declare_dram_parameter("condT", [D, NK], BF16, isOutput=False)
    wq = nc.declare_dram_parameter("wq", [D, D], BF16, isOutput=False)
    wk = nc.declare_dram_parameter("wk", [D, D], BF16, isOutput=False)
    wv = nc.declare_dram_parameter("wv", [D, D], BF16, isOutput=False)
    wo = nc.declare_dram_parameter("wo", [D, D], BF16, isOutput=False)
    bo = nc.declare_dram_parameter("bo", [1, D], F32, isOutput=False)
    out = nc.declare_dram_parameter("out", [IQ, D], F32, isOutput=True)

    gw = HPG * DH  # 256 columns of inner dim per group

    with tile.TileContext(nc) as tc:
        with (
            nc.allow_low_precision(reason="bf16 matmul operands are intended"),
            tc.tile_pool(name="const", bufs=1) as const,
            tc.tile_pool(name="big", bufs=1) as big,
            tc.tile_pool(name="grp", bufs=1) as grp,
            tc.tile_pool(name="expp", bufs=4) as expp,
            tc.tile_pool(name="small", bufs=1) as small,
            tc.tile_pool(name="ostage", bufs=2) as ostage,
            tc.tile_pool(name="ps", bufs=1, space="PSUM") as ps,
        ):
            bo_bc = const.tile([128, D], F32)
            nc.sync.dma_start(out=bo_bc, in_=bo[:, :].to_broadcast((128, D)))
            ones = const.tile([128, 128], BF16)
            nc.any.memset(ones, 1.0)

            def wq_tile(g):
                t = grp.tile([128, KT, gw], BF16, tag="wq_g", bufs=2,
                             name=f"wq_g{g}")
                for k in range(KT):
                    nc.sync.dma_start(
                        out=t[:, k, :],
                        in_=wq[k * 128:(k + 1) * 128, g * gw:(g + 1) * gw])
                return t

            def wk_tile(g):
                t = grp.tile([128, KT, gw], BF16, tag="wk_g", bufs=2,
                             name=f"wk_g{g}")
                for k in range(KT):
                    nc.sync.dma_start(
                        out=t[:, k, :],
                        in_=wk[k * 128:(k + 1) * 128, g * gw:(g + 1) * gw])
                return t

            # ---- prologue DMAs (issue order = DMA priority) ----
            wq_gs = {0: wq_tile(0)}
            xr = big.tile([128, KT, IQ], BF16)        # resident x^T
            for k in range(KT):
                nc.sync.dma_start(out=xr[:, k, :],
                                  in_=xT[k * 128:(k + 1) * 128, :])
            wk_gs = {0: wk_tile(0)}
            # cond^T and Wv ride the second HWDGE queue (Activation engine),
            # in parallel with the sync-queue loads above.
            ct = big.tile([128, KT, NK], BF16)        # resident cond^T
            for jh2 in range(2):   # first half of j range first (k-proj order)
                for k in range(KT):
                    nc.scalar.dma_start(
                        out=ct[:, k, jh2 * 1024:(jh2 + 1) * 1024],
                        in_=condT[k * 128:(k + 1) * 128,
                                  jh2 * 1024:(jh2 + 1) * 1024])
            wv_r = grp.tile([128, KT, D], BF16, tag="wvwo", name="wv_r")
            for k in range(KT):
                nc.scalar.dma_start(out=wv_r[:, k, :],
                                    in_=wv[k * 128:(k + 1) * 128, :])

            # attention output, transposed layout: [dh, head, i]
            attT = big.tile([128, H, IQ], BF16)
            # V for all heads: [j-part, jt, inner]
            v_all = big.tile([128, JT, D], BF16)

            qT_g = grp.tile([128, HPG, IQ], BF16, tag="qT_g", bufs=2)
            kT_g = grp.tile([128, HPG, NK], BF16, tag="kT_g", bufs=2)

            def pacc(nm):
                return ps.tile([128, 512], F32, tag="sc", bufs=2, name=nm)

            # ---- projection batches (~16 matmuls each, one stationary
            #      shared by consecutive matmul pairs) ----
            def q_batch(g, qT_t, wq_t, mh):
                accs = [pacc(f"q{g}_{mh}_{ih}") for ih in range(2)]
                for k in range(KT):
                    for ih in range(2):
                        nc.tensor.matmul(
                            accs[ih],
                            wq_t[:, k, mh * DH:(mh + 1) * DH],
                            xr[:, k, ih * 512:(ih + 1) * 512],
                            start=(k == 0), stop=(k == KT - 1))
                for ih in range(2):
                    nc.vector.tensor_copy(
                        qT_t[:, mh, ih * 512:(ih + 1) * 512], accs[ih])

            def k_batch(g, kT_t, wk_t, mh, jp):
                accs = [pacc(f"k{g}_{mh}_{jp}_{j}") for j in range(2)]
                for k in range(KT):
                    for j in range(2):
                        jh = jp * 2 + j
                        nc.tensor.matmul(
                            accs[j],
                            wk_t[:, k, mh * DH:(mh + 1) * DH],
                            ct[:, k, jh * 512:(jh + 1) * 512],
                            start=(k == 0), stop=(k == KT - 1))
                for j in range(2):
                    jh = jp * 2 + j
                    nc.vector.tensor_copy(
                        kT_t[:, mh, jh * 512:(jh + 1) * 512], accs[j])

            def qk_batches(g, qT_t, kT_t):
                fs = []
                for mh in range(HPG):
                    fs.append(lambda mh=mh: q_batch(g, qT_t, wq_gs[g], mh))
                for jp in range(2):
                    for mh in range(HPG):
                        fs.append(lambda mh=mh, jp=jp:
                                  k_batch(g, kT_t, wk_gs[g], mh, jp))
                return fs

            def v_batch(jt):
                accs = [pacc(f"v{jt}_{ch}") for ch in range(2)]
                for k in range(KT):
                    for ch in range(2):
                        nc.tensor.matmul(
                            accs[ch],
                            ct[:, k, jt * 128:(jt + 1) * 128],
                            wv_r[:, k, ch * 512:(ch + 1) * 512],
                            start=(k == 0), stop=(k == KT - 1))
                for ch in range(2):
                    nc.vector.tensor_copy(v_all[:, jt, ch * 512:(ch + 1) * 512],
                                          accs[ch])

            def q_proj(g, qT_t, wq_t):
                for mh in range(HPG):
                    q_batch(g, qT_t, wq_t, mh)

            def k_proj(g, kT_t, wk_t):
                for jp in range(2):
                    for mh in range(HPG):
                        k_batch(g, kT_t, wk_t, mh, jp)

            q_proj(0, qT_g, wq_gs[0])
            k_proj(0, kT_g, wk_gs[0])
            v_batch(0)
            v_batch(1)
            wq_gs[1] = wq_tile(1)
            wk_gs[1] = wk_tile(1)

            qkT = {0: (qT_g, kT_g)}

            def attention(g, feeders=()):
                """Heads A=2g, B=2g+1 interleaved over key tiles. `feeders`
                are projection batches for upcoming groups, interleaved one
                per key tile to keep the PE fed while the scalar engine
                paces the exps. Returns a closure that finishes the softmax
                (denominator reduce via ones-matmul on the PE, reciprocal,
                scale) — emitted after the next group's projections so the
                vector queue services those copies first."""
                qT_t, kT_t = qkT[g]
                avs = {(hh, i): ps.tile([128, 512], F32, tag="av", bufs=4,
                                        name=f"av_{g}_{hh}_{i}")
                       for hh in (0, 1) for i in range(2)}
                dens = {}
                for hh in (0, 1):
                    dens[hh] = small.tile([128, IQ], BF16, tag=f"den_{hh}",
                                          name=f"den_{g}_{hh}")
                escs = {}
                for jt in range(JT):
                    if jt < len(feeders):
                        feeders[jt]()
                    for hh in (0, 1):
                        sc = ps.tile([128, IQ], F32, tag="sc", bufs=2,
                                     name=f"sc_{g}_{hh}_{jt}")
                        for ih in range(2):
                            nc.tensor.matmul(
                                sc[:, ih * 512:(ih + 1) * 512],
                                kT_t[:, hh, jt * 128:(jt + 1) * 128],
                                qT_t[:, hh, ih * 512:(ih + 1) * 512],
                                start=True, stop=True)
                        esc = expp.tile([128, IQ], BF16, tag="esc",
                                        name=f"esc_{g}_{hh}_{jt}")
                        nc.scalar.activation(esc, sc, EXP)
                        escs[hh] = esc
                    for hh in (0, 1):
                        h = 2 * g + hh
                        esc = escs[hh]
                        for ih in range(2):
                            nc.tensor.matmul(
                                avs[(hh, ih)],
                                v_all[:, jt, h * DH:(h + 1) * DH],
                                esc[:, ih * 512:(ih + 1) * 512],
                                start=(jt == 0), stop=(jt == JT - 1))
                        with tc.high_priority():
                            if jt == 0:
                                nc.vector.tensor_copy(dens[hh], esc)
                            else:
                                nc.vector.tensor_add(dens[hh], dens[hh], esc)

                def finish():
                    for hh in (0, 1):
                        h = 2 * g + hh
                        den_ps = ps.tile([128, IQ], F32, tag="sc", bufs=2,
                                         name=f"den_ps_{g}_{hh}")
                        for ih in range(2):
                            nc.tensor.matmul(
                                den_ps[:, ih * 512:(ih + 1) * 512],
                                ones,
                                dens[hh][:, ih * 512:(ih + 1) * 512],
                                start=True, stop=True)
                        den_rc = small.tile([128, IQ], F32,
                                            tag=f"den_rc_{hh}",
                                            name=f"den_rc_{g}_{hh}")
                        nc.vector.reciprocal_approx_fast(out=den_rc,
                                                         in_=den_ps)
                        for ih in range(2):
                            nc.vector.tensor_mul(
                                attT[:, h, ih * 512:(ih + 1) * 512],
                                avs[(hh, ih)],
                                den_rc[:, ih * 512:(ih + 1) * 512])
                return finish

            # att(0) absorbs the remaining v projection (one v batch per key
            # tile, two tiles of lookahead); att(1)/att(2) absorb the q/k
            # projections of groups 2/3.
            finish = attention(0, [lambda jt=jt: v_batch(jt)
                                   for jt in range(2, JT)])
            # Wo reuses the Wv slot (Wv fully consumed by the v projection).
            wo_r = grp.tile([128, KT, D], BF16, tag="wvwo", name="wo_r")
            for k in range(KT):
                nc.sync.dma_start(out=wo_r[:, k, :],
                                  in_=wo[k * 128:(k + 1) * 128, :])
            qkT[1] = (grp.tile([128, HPG, IQ], BF16, tag="qT_g", bufs=2,
                               name="qT_g1"),
                      grp.tile([128, HPG, NK], BF16, tag="kT_g", bufs=2,
                               name="kT_g1"))
            q_proj(1, qkT[1][0], wq_gs[1])
            k_proj(1, qkT[1][1], wk_gs[1])
            wq_gs[2] = wq_tile(2)   # lands during this projection phase
            wk_gs[2] = wk_tile(2)
            finish()

            for g in (1, 2, 3):
                feeders = []
                if g + 1 < GROUPS:
                    if g + 2 < GROUPS:   # lands during att(g), used in att(g+1)
                        wq_gs[g + 2] = wq_tile(g + 2)
                        wk_gs[g + 2] = wk_tile(g + 2)
                    qT_t = grp.tile([128, HPG, IQ], BF16, tag="qT_g", bufs=2,
                                    name=f"qT_g{g + 1}")
                    kT_t = grp.tile([128, HPG, NK], BF16, tag="kT_g", bufs=2,
                                    name=f"kT_g{g + 1}")
                    qkT[g + 1] = (qT_t, kT_t)
                    feeders = qk_batches(g + 1, qT_t, kT_t)
                finish = attention(g, feeders)
                finish()

            # ---- output projection: out[it*128:, :] = attT.T @ Wo + bo ----
            for it in range(IQ // 128):
                accs = [pacc(f"o{it}_{nh}") for nh in range(2)]
                for k in range(KT):
                    for nh in range(2):
                        nc.tensor.matmul(
                            accs[nh],
                            attT[:, k, it * 128:(it + 1) * 128],
                            wo_r[:, k, nh * 512:(nh + 1) * 512],
                            start=(k == 0), stop=(k == KT - 1))
                for nh in range(2):
                    fo_sb = ostage.tile([128, 512], F32, tag="fo_sb",
                                        name=f"fo_{it}_{nh}")
                    nc.vector.tensor_add(fo_sb, accs[nh],
                                         bo_bc[:, nh * 512:(nh + 1) * 512])
                    nc.sync.dma_start(
                        out=out[it * 128:(it + 1) * 128,
                                nh * 512:(nh + 1) * 512],
                        in_=fo_sb)
    nc.finalize()
    return nc


_NC_CACHE = None


def _get_nc():
    global _NC_CACHE
    if _NC_CACHE is None:
        _NC_CACHE = build_nc()
    return _NC_CACHE


BF = ml_dtypes.bfloat16


def make_in_maps(x, cond, Wq, Wk, Wv, Wo, bo):
    wq_s = np.ascontiguousarray(Wq * SCALE).astype(BF)
    wk_c = np.ascontiguousarray(Wk).astype(BF)
    wv_c = np.ascontiguousarray(Wv).astype(BF)
    wo_c = np.ascontiguousarray(Wo).astype(BF)
    bo_c = np.ascontiguousarray(bo, dtype=np.float32).reshape(1, D)
    in_maps = []
    for c in range(NCORES):
        b, ih = c // 2, c % 2
        in_maps.append({
            "xT": np.ascontiguousarray(x[b, ih * IQ:(ih + 1) * IQ, :].T).astype(BF),
            "condT": np.ascontiguousarray(cond[b].T).astype(BF),
            "wq": wq_s, "wk": wk_c, "wv": wv_c, "wo": wo_c, "bo": bo_c,
        })
    return in_maps


def kernel(x, cond, Wq, Wk, Wv, Wo, bo, _trace=False, _trace_kwargs=None):
    x = np.asarray(x, dtype=np.float32)
    cond = np.asarray(cond, dtype=np.float32)
    nc = _get_nc()
    in_maps = make_in_maps(x, cond,
                           np.asarray(Wq, np.float32), np.asarray(Wk, np.float32),
                           np.asarray(Wv, np.float32), np.asarray(Wo, np.float32),
                           np.asarray(bo, np.float32))
    kw = {}
    if _trace:
        kw = {"trace": True, "trace_kwargs": _trace_kwargs or {}}
    res = run_bass_kernel_spmd(nc, in_maps, list(range(NCORES)), **kw)
    out = np.empty((B, NQ, D), dtype=np.float32)
    for c in range(NCORES):
        b, ih = c // 2, c % 2
        out[b, ih * IQ:(ih + 1) * IQ, :] = res.results[c]["out"]
    if _trace:
        return out, res
    return out


if __name__ == "__main__":
    # quick numeric self-check against numpy (no jax needed)
    rng = np.random.default_rng(0)
    s = 0.02
    x = rng.standard_normal((B, NQ, D), dtype=np.float32)
    cond = rng.standard_normal((B, NK, D), dtype=np.float32)
    Wq = (rng.standard_normal((D, D), dtype=np.float32) * s)
    Wk = (rng.standard_normal((D, D), dtype=np.float32) * s)
    Wv = (rng.standard_normal((D, D), dtype=np.float32) * s)
    Wo = (rng.standard_normal((D, D), dtype=np.float32) * s)
    bo = (rng.standard_normal((D,), dtype=np.float32) * s)

    def ref_np(x, cond):
        q = (x @ Wq).reshape(B, NQ, H, DH).transpose(0, 2, 1, 3)
        k = (cond @ Wk).reshape(B, NK, H, DH).transpose(0, 2, 1, 3)
        v = (cond @ Wv).reshape(B, NK, H, DH).transpose(0, 2, 1, 3)
        sim = np.einsum('bhid,bhjd->bhij', q, k) * SCALE
        sim = sim - sim.max(axis=-1, keepdims=True)
        a = np.exp(sim)
        a = a / a.sum(axis=-1, keepdims=True)
        o = np.einsum('bhij,bhjd->bhid', a, v)
        o = o.transpose(0, 2, 1, 3).reshape(B, NQ, D)
        return o @ Wo + bo

    import time
    t0 = time.time()
    got = kernel(x=x, cond=cond, Wq=Wq, Wk=Wk, Wv=Wv, Wo=Wo, bo=bo)
    print(f"kernel run {time.time()-t0:.1f}s")
    exp = ref_np(x.astype(np.float64), cond.astype(np.float64))
    err = np.abs(got - exp)
    rel = np.linalg.norm(got - exp) / np.linalg.norm(exp)
    print(f"rel_l2={rel:.3e} absmax_rel={err.max()/np.abs(exp).max():.3e}")
